# revision 16
# baseline (speedup 1.0000x reference)
"""Dense transformer block (QKV -> causal attention -> out-proj -> FFN+ReLU)
on 8 Trainium2 NeuronCores, data-parallel over the batch dimension.

Contract: kernel(**inputs) takes the FULL inputs
  x [8, 1024, 1024] f32, Wq/Wk/Wv/Wo/W1 [1024, 1024] f32, bo/b1 [1024] f32
and returns the FULL output [8, 1024, 1024] f32.

Each of the 8 cores runs the identical single-core program on one batch
element (batch=8, cores=8 -> no collectives needed).

Single-core design (bf16 tensor-engine compute, fp32 accumulation):
  - x arrives f32 on the two HWDGE queues (sync: chunks 0-3, scalar:
    chunks 4-7) so the SWDGE casting queue carries only the 20MB of
    weights; Wq lands ~17us in instead of ~35us.
  - x is PE-transposed from f32 (2 cyc/row) into bf16 feature-major
    xT [E, T]; the f32 transpose doubles as PE p-state warmup.
  - qT/kT produced feature-major per 512-token half; attention for
    t1=0 starts right after q/k half-0, streaming exp on the scalar
    engine while the PE continues qh1/kh1/v-proj as fillers.
  - scores for a head PAIR accumulate into one 2-bank PSUM tile
    [128, 2*512]; ONE exp ACTIVATE covers both heads (halves the
    per-instruction ACT overhead). attnv lags scores by LAG pairs
    (pt tiles buffer the exp'd scores) so Wv's DMA arrival never
    stalls the exp stream.
  - v is token-major "augmented": each head owns a 128-col block
    [ones col | 63 zero cols | 64 value cols] so the attn@v PSUM
    carries the softmax sum at row 0 and values at rows 64..127.
  - normalization: reciprocal_approx_fast from PSUM row 0 (DVE),
    partition_broadcast on gpsimd (no DMA traffic), tensor_mul (DVE).
  - out-proj bias via DVE tensor_scalar_add; FFN bias via a K=1
    ones-row matmul; ReLU on the scalar engine evicting to bf16.
  - output is stored bf16 (host casts back to f32; well within the
    relative-error budget) with stores split across sync/scalar/
    gpsimd queues so the tail drains fast.
"""

import numpy as np
from contextlib import ExitStack

import concourse.bass as bass
import concourse.bacc as bacc
import concourse.tile as tile
from concourse import mybir
from concourse.bass_utils import run_bass_kernel_spmd

F32 = mybir.dt.float32
BF16 = mybir.dt.bfloat16

N_CORES = 8
BATCH = 8
T = 1024
E = 1024
H = 16
DH = 64


def build_nc(TT=T, EE=E, HH=H, Dh=DH):
    nc = bacc.Bacc("TRN2", target_bir_lowering=False, num_swdge_queues=4)

    x = nc.dram_tensor("x", [TT, EE], F32, kind="ExternalInput")
    Wq = nc.dram_tensor("Wq", [EE, EE], F32, kind="ExternalInput")
    Wk = nc.dram_tensor("Wk", [EE, EE], F32, kind="ExternalInput")
    Wv = nc.dram_tensor("Wv", [EE, EE], F32, kind="ExternalInput")
    Wo = nc.dram_tensor("Wo", [EE, EE], F32, kind="ExternalInput")
    bo = nc.dram_tensor("bo", [EE], F32, kind="ExternalInput")
    W1 = nc.dram_tensor("W1", [EE, EE], F32, kind="ExternalInput")
    b1 = nc.dram_tensor("b1", [EE], F32, kind="ExternalInput")
    out = nc.dram_tensor("out", [TT, EE], BF16, kind="ExternalOutput")

    EC = EE // 128          # feature-chunk count (partition tiles)
    TC = TT // 128          # token-chunk count
    QT = min(512, TT)       # t1 (query) free-dim chunk
    NT = TT // QT
    QE = min(512, EE)       # output-feature free-dim chunk
    NE = EE // QE
    HP = 128 // Dh          # heads per 128-partition feature tile
    NP = HH // HP           # number of head pairs (= EC)
    LAG = 1                 # attnv lags scores by this many pairs
    scale = float(Dh) ** -0.5
    Exp = mybir.ActivationFunctionType.Exp
    Relu = mybir.ActivationFunctionType.Relu

    with ExitStack() as ctx:
        tc = ctx.enter_context(tile.TileContext(nc))
        wpool = ctx.enter_context(tc.tile_pool(name="w", bufs=3 * EC))
        xtokp = ctx.enter_context(tc.tile_pool(name="xtok", bufs=3))
        xTp = ctx.enter_context(tc.tile_pool(name="xT", bufs=EC))
        qTp = ctx.enter_context(tc.tile_pool(name="qT", bufs=EC))
        kTp = ctx.enter_context(tc.tile_pool(name="kT", bufs=EC))
        vp = ctx.enter_context(tc.tile_pool(name="v", bufs=TC))
        pp = ctx.enter_context(tc.tile_pool(name="p", bufs=10))
        rtp = ctx.enter_context(tc.tile_pool(name="rt", bufs=2))
        rbp = ctx.enter_context(tc.tile_pool(name="rb", bufs=2))
        aoutp = ctx.enter_context(tc.tile_pool(name="aout", bufs=EC))
        projp = ctx.enter_context(tc.tile_pool(name="proj", bufs=EC))
        constp = ctx.enter_context(tc.tile_pool(name="const", bufs=1))
        ffoutp = ctx.enter_context(tc.tile_pool(name="ffout", bufs=2))
        ps_acc = ctx.enter_context(tc.tile_pool(name="ps_acc", bufs=2, space="PSUM"))
        ps_s = ctx.enter_context(tc.tile_pool(name="ps_s", bufs=2, space="PSUM"))
        ps_o = ctx.enter_context(tc.tile_pool(name="ps_o", bufs=2, space="PSUM"))

        # ---- constants ----
        bo_sb = constp.tile([128, EC], F32)
        nc.sync.dma_start(out=bo_sb, in_=bo.rearrange("(c p) -> p c", p=128))
        b1_sb = constp.tile([1, EE], BF16)
        ones_t = constp.tile([1, 128], BF16)
        nc.vector.memset(ones_t, 1.0)
        ident = constp.tile([128, 128], BF16)
        identf = constp.tile([128, 128], F32)
        from concourse.masks import make_identity
        make_identity(nc, ident)
        nc.vector.tensor_copy(out=identf, in_=ident)
        # causal mask as a PSUM-accumulated matmul: matmul(lhsT=A, rhs=ident)
        # adds A^T to the score block. We want score[p, c] += -30000 where
        # c < p (future keys), so A[k, m] = -30000 where k < m: keep where
        # (c - p) <= 0, fill the rest.
        tri_negT = constp.tile([128, 128], BF16)
        nc.gpsimd.memset(tri_negT, 0.0)
        nc.gpsimd.affine_select(
            out=tri_negT, in_=tri_negT,
            compare_op=mybir.AluOpType.is_ge,
            fill=-30000.0, base=0, pattern=[[-1, 128]], channel_multiplier=1,
        )

        # PE warm-up: the tensor engine's clock ramps to full speed only
        # after ~3us of continuous execution. Burn the x-DMA lead-in on
        # dummy transposes of the identity constant.
        for _ in range(12):
            wps = ps_s.tile([128, 128], BF16, name="warm", tag="sp")
            nc.tensor.transpose(wps, ident, ident)

        # ---- x: f32 on the two HWDGE queues, then PE-transpose ----
        xT = [xTp.tile([128, TT], BF16, name="xT", tag="xT") for _ in range(EC)]
        xtoks = []
        for ti in range(TC):
            xtok = xtokp.tile([128, EE], F32, tag="xtok")
            eng = nc.sync if ti < TC // 2 else nc.scalar
            if ti == 0:
                # split the first chunk so transposes start after a quarter
                q4 = EE // 4
                for s in range(4):
                    nc.sync.dma_start(
                        out=xtok[:, q4 * s:q4 * (s + 1)],
                        in_=x[0:128, q4 * s:q4 * (s + 1)],
                    )
            else:
                eng.dma_start(out=xtok, in_=x[128 * ti:128 * (ti + 1), :])
            xtoks.append(xtok)

        def load_w(wdram):
            tiles = []
            for ei in range(EC):
                wt = wpool.tile([128, EE], BF16, tag="w")
                nc.gpsimd.dma_start(out=wt, in_=wdram[128 * ei:128 * (ei + 1), :])
                tiles.append(wt)
            return tiles

        # weights ride the SWDGE casting queue alone, in consumption order
        wq = load_w(Wq)
        wk = load_w(Wk)
        wv = load_w(Wv)
        nc.gpsimd.dma_start(out=b1_sb, in_=b1.rearrange("(a e) -> a e", a=1))
        wo = load_w(Wo)
        w1 = load_w(W1)

        def emit_xpose(tis, use_scalar=True):
            for ti in tis:
                for ec in range(EC):
                    ps_t = ps_acc.tile([128, 128], F32, name="ps_t", tag="ps_acc")
                    nc.tensor.transpose(
                        ps_t, xtoks[ti][:, 128 * ec:128 * (ec + 1)], identf
                    )
                    dst = xT[ec][:, 128 * ti:128 * (ti + 1)]
                    if use_scalar and ec % 2 == 1:
                        nc.scalar.copy(out=dst, in_=ps_t)
                    else:
                        nc.vector.tensor_copy(out=dst, in_=ps_t)

        # ---- q/k: feature-major [128, T] per chunk, per t1 half ----
        qT = [qTp.tile([128, TT], BF16, name="qT", tag="qT") for _ in range(EC)]
        kT = [kTp.tile([128, TT], BF16, name="kT", tag="kT") for _ in range(EC)]

        def emit_proj_half(wtiles, dst, t1, evict, sink=None):
            for eo in range(EC):
                box = {}

                def mm(ei, eo=eo, box=box):
                    if ei == 0:
                        box["ps"] = ps_acc.tile(
                            [128, QT], F32, name="ps_acc", tag="ps_acc"
                        )
                    nc.tensor.matmul(
                        box["ps"],
                        lhsT=wtiles[ei][:, 128 * eo:128 * (eo + 1)],
                        rhs=xT[ei][:, QT * t1:QT * (t1 + 1)],
                        start=(ei == 0),
                        stop=(ei == EC - 1),
                    )

                def ev(eo=eo, box=box):
                    evict(out=dst[eo][:, QT * t1:QT * (t1 + 1)], in_=box["ps"])

                thunks = [lambda ei=ei, mm=mm: mm(ei) for ei in range(EC)] + [ev]
                if sink is None:
                    for t in thunks:
                        t()
                else:
                    sink.extend(thunks)

        # ---- v: token-major augmented, one 128-col block per head ----
        vaug = [None] * TC

        # augmented-v block width per head: [ones col | VZ-1 zero cols |
        # Dh value cols]; VZ=64 keeps the value rows at partition 64
        # (DVE patterns wider than 32 partitions must start at 0 or 64).
        VZ = 64
        VW = VZ + Dh

        def emit_vchunk(ti, sink=None):
            va = vp.tile([128, VW * HH], BF16, name="va")
            ones_ap = bass.AP(
                tensor=va.tensor, offset=va.offset,
                ap=[list(va.ap[0]), [VW, HH], [1, 1]],
            )
            nc.gpsimd.memset(ones_ap, 1.0)
            zeros_ap = bass.AP(
                tensor=va.tensor, offset=va.offset + 1,
                ap=[list(va.ap[0]), [VW, HH], [1, VZ - 1]],
            )
            nc.gpsimd.memset(zeros_ap, 0.0)
            vaug[ti] = va
            for eoq in range(NE):
                box = {}

                def mm(ei, eoq=eoq, box=box):
                    if ei == 0:
                        box["ps"] = ps_acc.tile(
                            [128, QE], F32, name="ps_acc", tag="ps_acc"
                        )
                    nc.tensor.matmul(
                        box["ps"],
                        lhsT=xT[ei][:, 128 * ti:128 * (ti + 1)],
                        rhs=wv[ei][:, QE * eoq:QE * (eoq + 1)],
                        start=(ei == 0),
                        stop=(ei == EC - 1),
                    )

                def ev(eoq=eoq, box=box):
                    hq = QE // Dh
                    dst = va[:, VW * hq * eoq:VW * hq * (eoq + 1)]
                    dst = dst.rearrange("p (h c) -> p h c", c=VW)[:, :, VZ:VW]
                    src = box["ps"].rearrange("p (h d) -> p h d", d=Dh)
                    nc.scalar.copy(out=dst, in_=src)

                thunks = [lambda ei=ei, mm=mm: mm(ei) for ei in range(EC)] + [ev]
                if sink is None:
                    for t in thunks:
                        t()
                else:
                    sink.extend(thunks)

        # ---- attention building blocks ----
        aoutT = [aoutp.tile([128, TT], BF16, name="aoutT", tag="aoutT") for _ in range(EC)]
        projT = [projp.tile([128, TT], BF16, name="projT", tag="projT") for _ in range(EC)]

        def emit_scores(p, t1, t2cs):
            """Scores + mask + ONE pair-wide exp per t2 unit. Returns pts."""
            pts = []
            for t2 in t2cs:
                k0 = 128 * t2 - QT * t1
                c0 = max(0, k0)
                diag = k0 >= 0
                sp2 = ps_s.tile([128, 2 * QT], F32, name="sp2", tag="sp")
                for hi in range(HP):
                    po = hi * Dh
                    nc.tensor.matmul(
                        sp2[:, QT * hi + c0:QT * (hi + 1)],
                        lhsT=kT[p][po:po + Dh, 128 * t2:128 * (t2 + 1)],
                        rhs=qT[p][po:po + Dh, QT * t1 + c0:QT * (t1 + 1)],
                        start=True,
                        stop=not diag,
                    )
                if diag:
                    for hi in range(HP):
                        nc.tensor.matmul(
                            sp2[:, QT * hi + c0:QT * hi + c0 + 128],
                            lhsT=tri_negT,
                            rhs=ident,
                            start=False,
                            stop=True,
                        )
                pt = pp.tile([128, 2 * QT], BF16)
                src = sp2.rearrange("p (h f) -> p h f", h=HP)[:, :, c0:QT]
                dst = pt.rearrange("p (h f) -> p h f", h=HP)[:, :, c0:QT]
                nc.scalar.activation(out=dst, in_=src, func=Exp, scale=scale)
                pts.append((t2, c0, pt))
            return pts

        def emit_attnv(p, t1, pts):
            opss = [ps_o.tile([128, QT], F32, name="ops", tag="ops")
                    for _ in range(HP)]
            n = len(pts)
            for j, (t2, c0, pt) in enumerate(pts):
                for hi in range(HP):
                    h = HP * p + hi
                    nc.tensor.matmul(
                        opss[hi][0:VW, c0:QT],
                        lhsT=vaug[t2][:, VW * h:VW * (h + 1)],
                        rhs=pt[:, QT * hi + c0:QT * (hi + 1)],
                        start=(j == 0),
                        stop=(j == n - 1),
                    )
            # normalization: recip of PSUM row 0, gpsimd partition
            # broadcast (no DMA), fused evict-multiply to bf16 SBUF.
            rtss = [rtp.tile([1, QT], F32, name="rts", tag="rts")
                    for _ in range(HP)]
            rb = rbp.tile([128, QT], F32)
            for hi in range(HP):
                nc.vector.reciprocal_approx_fast(
                    out=rtss[hi], in_=opss[hi][0:1, :],
                )
            for hi in range(HP):
                po = hi * Dh
                r_h = rtss[hi]
                r_src = bass.AP(
                    tensor=r_h.tensor,
                    offset=r_h.offset,
                    ap=[list(r_h.ap[0]), [0, Dh]] + list(r_h.ap[1:]),
                )
                nc.sync.dma_start(out=rb[po:po + Dh, :], in_=r_src)
                nc.vector.tensor_mul(
                    out=aoutT[p][po:po + Dh, QT * t1:QT * (t1 + 1)],
                    in0=opss[hi][VZ:VW, :],
                    in1=rb[po:po + Dh, :],
                )

        def emit_outproj(eo, t1, sink=None, pool=None, tag="ps_acc"):
            box = {}

            def mm(ei):
                if ei == 0:
                    box["ps"] = (pool or ps_acc).tile(
                        [128, QT], F32, name="ps_acc", tag=tag
                    )
                nc.tensor.matmul(
                    box["ps"],
                    lhsT=wo[ei][:, 128 * eo:128 * (eo + 1)],
                    rhs=aoutT[ei][:, QT * t1:QT * (t1 + 1)],
                    start=(ei == 0),
                    stop=(ei == EC - 1),
                )

            def ev():
                nc.vector.tensor_scalar_add(
                    out=projT[eo][:, QT * t1:QT * (t1 + 1)],
                    in0=box["ps"],
                    scalar1=bo_sb[:, eo:eo + 1],
                )

            thunks = [lambda ei=ei: mm(ei) for ei in range(EC)] + [ev]
            if sink is None:
                for t in thunks:
                    t()
            else:
                sink.extend(thunks)

        store_rr = [0]
        store_engs = [nc.sync, nc.scalar, nc.gpsimd]

        def emit_ffn(ti, sink=None, pool=None, tag="ps_acc"):
            for eoq in range(NE):
                box = {}

                def mm(ei, eoq=eoq, box=box):
                    if ei == 0:
                        box["ps"] = (pool or ps_acc).tile(
                            [128, QE], F32, name="ps_acc", tag=tag
                        )
                    nc.tensor.matmul(
                        box["ps"],
                        lhsT=projT[ei][:, 128 * ti:128 * (ti + 1)],
                        rhs=w1[ei][:, QE * eoq:QE * (eoq + 1)],
                        start=(ei == 0),
                        stop=False,
                    )

                def bias(eoq=eoq, box=box):
                    nc.tensor.matmul(
                        box["ps"],
                        lhsT=ones_t[:, 0:128],
                        rhs=b1_sb[:, QE * eoq:QE * (eoq + 1)],
                        start=False,
                        stop=True,
                    )

                def ev(eoq=eoq, box=box):
                    fo = ffoutp.tile([128, QE], BF16)
                    nc.scalar.activation(out=fo, in_=box["ps"], func=Relu)
                    h0 = QE // 2
                    for s in range(2):
                        eng = store_engs[store_rr[0] % 3]
                        store_rr[0] += 1
                        eng.dma_start(
                            out=out[128 * ti:128 * (ti + 1),
                                    QE * eoq + s * h0:QE * eoq + (s + 1) * h0],
                            in_=fo[:, s * h0:(s + 1) * h0],
                        )

                thunks = [lambda ei=ei, mm=mm: mm(ei) for ei in range(EC)] + [bias, ev]
                if sink is None:
                    for t in thunks:
                        t()
                else:
                    sink.extend(thunks)

        # ================= schedule =================
        emit_xpose(range(TC // 2))
        emit_proj_half(
            wq, qT, 0, lambda out, in_: nc.vector.tensor_copy(out=out, in_=in_)
        )
        emit_proj_half(
            wk, kT, 0, lambda out, in_: nc.scalar.copy(out=out, in_=in_)
        )
        # x chunks 4-7 transposed before v (v chunk ti only needs token
        # chunk ti, but q/k half-1 fillers below need all of xT).
        emit_xpose(range(TC // 2, TC))
        # v chunks 0-3 emitted inline BEFORE the attention loop: the
        # lagged attnv(0) below must come after v3's eviction in program
        # order. The PE reaches here at ~50us, right as Wv's DMA lands.
        for ti in range(QT // 128):
            emit_vchunk(ti)

        # ---- attention t1=0 with lagged attnv and PE fillers ----
        # Fillers: q half-1 and k half-1 (no consumer inside t1=0).
        fillers = []
        emit_proj_half(
            wq, qT, 1,
            lambda out, in_: nc.vector.tensor_copy(out=out, in_=in_),
            sink=fillers,
        )
        emit_proj_half(
            wk, kT, 1,
            lambda out, in_: nc.vector.tensor_copy(out=out, in_=in_),
            sink=fillers,
        )

        t2cs0 = [t2 for t2 in range(TC) if 128 * t2 < QT]
        pending = {}
        fidx = [0]

        def drain_fillers(n):
            k = fidx[0]
            stop = min(len(fillers), k + n)
            while k < stop:
                fillers[k]()
                k += 1
            fidx[0] = k

        for p in range(NP):
            pending[p] = emit_scores(p, 0, t2cs0)
            # keep the PE fed while ACT chews on this pair's exps
            drain_fillers(18)
            if p >= LAG:
                emit_attnv(p - LAG, 0, pending.pop(p - LAG))
        drain_fillers(len(fillers))
        for p in range(NP - LAG, NP):
            emit_attnv(p, 0, pending.pop(p))

        # v chunks 4-7 (needed by attention t1=1)
        for ti in range(QT // 128, TC):
            emit_vchunk(ti)

        # ---- attention t1=1 with out-proj t1=0 interleaved ----
        t2cs1 = list(range(TC))
        fillers = []
        for eo in range(EC):
            emit_outproj(eo, 0, sink=fillers)
        fidx = [0]
        pending = {}
        for p in range(NP):
            pending[p] = emit_scores(p, 1, t2cs1)
            drain_fillers(10)
            if p >= 1:
                emit_attnv(p - 1, 1, pending.pop(p - 1))
        drain_fillers(len(fillers))
        emit_attnv(NP - 1, 1, pending.pop(NP - 1))

        # ---- FFN t1=0 chunks interleaved with out-proj t1=1 ----
        pools = [(ps_acc, "ps_acc"), (ps_s, "sp"), (ps_o, "ops")]

        def pl(i):
            return pools[i % 3]

        t1l = NT - 1
        for i, ti in enumerate(range(QT // 128)):
            po, tg = pl(i)
            emit_ffn(ti, pool=po, tag=tg)
            po, tg = pl(i + 1)
            emit_outproj(2 * i, t1l, pool=po, tag=tg)
            po, tg = pl(i + 2)
            emit_outproj(2 * i + 1, t1l, pool=po, tag=tg)
        for i, ti in enumerate(range(QT // 128, TC)):
            po, tg = pl(i)
            emit_ffn(ti, pool=po, tag=tg)

    nc.finalize()
    return nc


_NC_CACHE = {}


def _get_nc(shape_key):
    if shape_key not in _NC_CACHE:
        _NC_CACHE[shape_key] = build_nc(*shape_key)
    return _NC_CACHE[shape_key]


def kernel(x, Wq, Wk, Wv, Wo, bo, W1, b1):
    x = np.ascontiguousarray(np.asarray(x, dtype=np.float32))
    ws = {
        "Wq": np.ascontiguousarray(np.asarray(Wq, dtype=np.float32)),
        "Wk": np.ascontiguousarray(np.asarray(Wk, dtype=np.float32)),
        "Wv": np.ascontiguousarray(np.asarray(Wv, dtype=np.float32)),
        "Wo": np.ascontiguousarray(np.asarray(Wo, dtype=np.float32)),
        "bo": np.ascontiguousarray(np.asarray(bo, dtype=np.float32)),
        "W1": np.ascontiguousarray(np.asarray(W1, dtype=np.float32)),
        "b1": np.ascontiguousarray(np.asarray(b1, dtype=np.float32)),
    }
    B, TT, EE = x.shape
    assert B == N_CORES
    nc = _get_nc((TT, EE, H, DH))
    in_maps = [dict(ws, x=x[b]) for b in range(B)]
    res = run_bass_kernel_spmd(nc, in_maps, core_ids=list(range(N_CORES)))
    return np.stack(
        [np.asarray(res.results[b]["out"]) for b in range(B)], axis=0
    ).astype(np.float32)


# revision 19
# speedup vs baseline: 1.4751x; 1.4751x over previous
"""Dense transformer block (QKV -> causal attention -> out-proj -> FFN+ReLU)
on 8 Trainium2 NeuronCores, data-parallel over the batch dimension.

Contract: kernel(**inputs) takes the FULL inputs
  x [8, 1024, 1024] f32, Wq/Wk/Wv/Wo/W1 [1024, 1024] f32, bo/b1 [1024] f32
and returns the FULL output [8, 1024, 1024] f32.

Each of the 8 cores runs the identical single-core program on one batch
element (batch=8, cores=8 -> no collectives needed).

Single-core design (bf16 tensor-engine compute, fp32 accumulation):
  - x arrives f32 on the two HWDGE queues (sync: chunks 0-3, scalar:
    chunks 4-7) so the SWDGE casting queue carries only the 20MB of
    weights; Wq lands ~17us in instead of ~35us.
  - x is PE-transposed from f32 (2 cyc/row) into bf16 feature-major
    xT [E, T]; the f32 transpose doubles as PE p-state warmup.
  - qT/kT produced feature-major per 512-token half; attention for
    t1=0 starts right after q/k half-0, streaming exp on the scalar
    engine while the PE continues qh1/kh1/v-proj as fillers.
  - scores for a head PAIR accumulate into one 2-bank PSUM tile
    [128, 2*512]; ONE exp ACTIVATE covers both heads (halves the
    per-instruction ACT overhead). attnv lags scores by LAG pairs
    (pt tiles buffer the exp'd scores) so Wv's DMA arrival never
    stalls the exp stream.
  - v is token-major "augmented": each head owns a 128-col block
    [ones col | 63 zero cols | 64 value cols] so the attn@v PSUM
    carries the softmax sum at row 0 and values at rows 64..127.
  - normalization: reciprocal_approx_fast from PSUM row 0 (DVE),
    partition_broadcast on gpsimd (no DMA traffic), tensor_mul (DVE).
  - out-proj bias via DVE tensor_scalar_add; FFN bias via a K=1
    ones-row matmul; ReLU on the scalar engine evicting to bf16.
  - output is stored bf16 (host casts back to f32; well within the
    relative-error budget) with stores split across sync/scalar/
    gpsimd queues so the tail drains fast.
"""

import numpy as np
from contextlib import ExitStack

import concourse.bass as bass
import concourse.bacc as bacc
import concourse.tile as tile
from concourse import mybir
from concourse.bass_utils import run_bass_kernel_spmd

F32 = mybir.dt.float32
BF16 = mybir.dt.bfloat16

N_CORES = 8
BATCH = 8
T = 1024
E = 1024
H = 16
DH = 64


def build_nc(TT=T, EE=E, HH=H, Dh=DH):
    nc = bacc.Bacc("TRN2", target_bir_lowering=False, num_swdge_queues=4)

    x = nc.dram_tensor("x", [TT, EE], F32, kind="ExternalInput")
    Wq = nc.dram_tensor("Wq", [EE, EE], F32, kind="ExternalInput")
    Wk = nc.dram_tensor("Wk", [EE, EE], F32, kind="ExternalInput")
    Wv = nc.dram_tensor("Wv", [EE, EE], F32, kind="ExternalInput")
    Wo = nc.dram_tensor("Wo", [EE, EE], F32, kind="ExternalInput")
    bo = nc.dram_tensor("bo", [EE], F32, kind="ExternalInput")
    W1 = nc.dram_tensor("W1", [EE, EE], F32, kind="ExternalInput")
    b1 = nc.dram_tensor("b1", [EE], F32, kind="ExternalInput")
    out = nc.dram_tensor("out", [TT, EE], BF16, kind="ExternalOutput")

    EC = EE // 128          # feature-chunk count (partition tiles)
    TC = TT // 128          # token-chunk count
    QT = min(512, TT)       # t1 (query) free-dim chunk
    NT = TT // QT
    QE = min(512, EE)       # output-feature free-dim chunk
    NE = EE // QE
    HP = 128 // Dh          # heads per 128-partition feature tile
    NP = HH // HP           # number of head pairs (= EC)
    LAG = 1                 # attnv lags scores by this many pairs
    scale = float(Dh) ** -0.5
    Exp = mybir.ActivationFunctionType.Exp
    Relu = mybir.ActivationFunctionType.Relu

    with ExitStack() as ctx:
        tc = ctx.enter_context(tile.TileContext(nc))
        wpool = ctx.enter_context(tc.tile_pool(name="w", bufs=3 * EC))
        xtokp = ctx.enter_context(tc.tile_pool(name="xtok", bufs=3))
        xTp = ctx.enter_context(tc.tile_pool(name="xT", bufs=EC))
        qTp = ctx.enter_context(tc.tile_pool(name="qT", bufs=EC))
        kTp = ctx.enter_context(tc.tile_pool(name="kT", bufs=EC))
        vp = ctx.enter_context(tc.tile_pool(name="v", bufs=TC))
        pp = ctx.enter_context(tc.tile_pool(name="p", bufs=10))
        rtp = ctx.enter_context(tc.tile_pool(name="rt", bufs=2))
        rbp = ctx.enter_context(tc.tile_pool(name="rb", bufs=3))
        aoutp = ctx.enter_context(tc.tile_pool(name="aout", bufs=EC))
        projp = ctx.enter_context(tc.tile_pool(name="proj", bufs=EC))
        constp = ctx.enter_context(tc.tile_pool(name="const", bufs=1))
        ffoutp = ctx.enter_context(tc.tile_pool(name="ffout", bufs=2))
        ps_acc = ctx.enter_context(tc.tile_pool(name="ps_acc", bufs=2, space="PSUM"))
        ps_s = ctx.enter_context(tc.tile_pool(name="ps_s", bufs=2, space="PSUM"))
        ps_o = ctx.enter_context(tc.tile_pool(name="ps_o", bufs=2, space="PSUM"))

        # ---- constants ----
        bo_sb = constp.tile([128, EC], F32)
        nc.sync.dma_start(out=bo_sb, in_=bo.rearrange("(c p) -> p c", p=128))
        b1_sb = constp.tile([1, EE], BF16)
        ones_t = constp.tile([1, 128], BF16)
        nc.vector.memset(ones_t, 1.0)
        ident = constp.tile([128, 128], BF16)
        identf = constp.tile([128, 128], F32)
        from concourse.masks import make_identity
        make_identity(nc, ident)
        nc.vector.tensor_copy(out=identf, in_=ident)
        # causal mask as a PSUM-accumulated matmul: matmul(lhsT=A, rhs=ident)
        # adds A^T to the score block. We want score[p, c] += -30000 where
        # c < p (future keys), so A[k, m] = -30000 where k < m: keep where
        # (c - p) <= 0, fill the rest.
        tri_negT = constp.tile([128, 128], BF16)
        nc.gpsimd.memset(tri_negT, 0.0)
        nc.gpsimd.affine_select(
            out=tri_negT, in_=tri_negT,
            compare_op=mybir.AluOpType.is_ge,
            fill=-30000.0, base=0, pattern=[[-1, 128]], channel_multiplier=1,
        )

        # PE warm-up: the tensor engine's clock ramps to full speed only
        # after ~3us of continuous execution. Burn the x-DMA lead-in on
        # dummy transposes of the identity constant.
        for _ in range(12):
            wps = ps_s.tile([128, 128], BF16, name="warm", tag="sp")
            nc.tensor.transpose(wps, ident, ident)

        # ---- x: f32 on the two HWDGE queues, then PE-transpose ----
        xT = [xTp.tile([128, TT], BF16, name="xT", tag="xT") for _ in range(EC)]
        xtoks = []
        for ti in range(TC):
            xtok = xtokp.tile([128, EE], F32, tag="xtok")
            eng = nc.sync if ti < TC // 2 else nc.scalar
            if ti == 0:
                # split the first chunk so transposes start after a quarter
                q4 = EE // 4
                for s in range(4):
                    nc.sync.dma_start(
                        out=xtok[:, q4 * s:q4 * (s + 1)],
                        in_=x[0:128, q4 * s:q4 * (s + 1)],
                    )
            else:
                eng.dma_start(out=xtok, in_=x[128 * ti:128 * (ti + 1), :])
            xtoks.append(xtok)

        def load_w(wdram):
            tiles = []
            for ei in range(EC):
                wt = wpool.tile([128, EE], BF16, tag="w")
                nc.gpsimd.dma_start(out=wt, in_=wdram[128 * ei:128 * (ei + 1), :])
                tiles.append(wt)
            return tiles

        # weights ride the SWDGE casting queue alone, in consumption order
        wq = load_w(Wq)
        wk = load_w(Wk)
        wv = load_w(Wv)
        nc.gpsimd.dma_start(out=b1_sb, in_=b1.rearrange("(a e) -> a e", a=1))
        wo = load_w(Wo)
        w1 = load_w(W1)

        def emit_xpose(tis, use_scalar=True):
            for ti in tis:
                for ec in range(EC):
                    ps_t = ps_acc.tile([128, 128], F32, name="ps_t", tag="ps_acc")
                    nc.tensor.transpose(
                        ps_t, xtoks[ti][:, 128 * ec:128 * (ec + 1)], identf
                    )
                    dst = xT[ec][:, 128 * ti:128 * (ti + 1)]
                    if use_scalar and ec % 2 == 1:
                        nc.scalar.copy(out=dst, in_=ps_t)
                    else:
                        nc.vector.tensor_copy(out=dst, in_=ps_t)

        # ---- q/k: feature-major [128, T] per chunk, per t1 half ----
        qT = [qTp.tile([128, TT], BF16, name="qT", tag="qT") for _ in range(EC)]
        kT = [kTp.tile([128, TT], BF16, name="kT", tag="kT") for _ in range(EC)]

        def emit_proj_half(wtiles, dst, t1, evict, sink=None):
            for eo in range(EC):
                box = {}

                def mm(ei, eo=eo, box=box):
                    if ei == 0:
                        box["ps"] = ps_acc.tile(
                            [128, QT], F32, name="ps_acc", tag="ps_acc"
                        )
                    nc.tensor.matmul(
                        box["ps"],
                        lhsT=wtiles[ei][:, 128 * eo:128 * (eo + 1)],
                        rhs=xT[ei][:, QT * t1:QT * (t1 + 1)],
                        start=(ei == 0),
                        stop=(ei == EC - 1),
                    )

                def ev(eo=eo, box=box):
                    evict(out=dst[eo][:, QT * t1:QT * (t1 + 1)], in_=box["ps"])

                thunks = [lambda ei=ei, mm=mm: mm(ei) for ei in range(EC)] + [ev]
                if sink is None:
                    for t in thunks:
                        t()
                else:
                    sink.extend(thunks)

        # ---- v: token-major augmented, one 128-col block per head ----
        vaug = [None] * TC

        # augmented-v block width per head: [ones col | VZ-1 zero cols |
        # Dh value cols]; VZ=64 keeps the value rows at partition 64
        # (DVE patterns wider than 32 partitions must start at 0 or 64).
        VZ = 64
        VW = VZ + Dh

        def emit_vchunk(ti, sink=None):
            va = vp.tile([128, VW * HH], BF16, name="va")
            ones_ap = bass.AP(
                tensor=va.tensor, offset=va.offset,
                ap=[list(va.ap[0]), [VW, HH], [1, 1]],
            )
            nc.gpsimd.memset(ones_ap, 1.0)
            zeros_ap = bass.AP(
                tensor=va.tensor, offset=va.offset + 1,
                ap=[list(va.ap[0]), [VW, HH], [1, VZ - 1]],
            )
            nc.gpsimd.memset(zeros_ap, 0.0)
            vaug[ti] = va
            for eoq in range(NE):
                box = {}

                def mm(ei, eoq=eoq, box=box):
                    if ei == 0:
                        box["ps"] = ps_acc.tile(
                            [128, QE], F32, name="ps_acc", tag="ps_acc"
                        )
                    nc.tensor.matmul(
                        box["ps"],
                        lhsT=xT[ei][:, 128 * ti:128 * (ti + 1)],
                        rhs=wv[ei][:, QE * eoq:QE * (eoq + 1)],
                        start=(ei == 0),
                        stop=(ei == EC - 1),
                    )

                def ev(eoq=eoq, box=box):
                    hq = QE // Dh
                    dst = va[:, VW * hq * eoq:VW * hq * (eoq + 1)]
                    dst = dst.rearrange("p (h c) -> p h c", c=VW)[:, :, VZ:VW]
                    src = box["ps"].rearrange("p (h d) -> p h d", d=Dh)
                    nc.scalar.copy(out=dst, in_=src)

                thunks = [lambda ei=ei, mm=mm: mm(ei) for ei in range(EC)] + [ev]
                if sink is None:
                    for t in thunks:
                        t()
                else:
                    sink.extend(thunks)

        # ---- attention building blocks ----
        aoutT = [aoutp.tile([128, TT], BF16, name="aoutT", tag="aoutT") for _ in range(EC)]
        projT = [projp.tile([128, TT], BF16, name="projT", tag="projT") for _ in range(EC)]

        def emit_scores(p, t1, t2cs):
            """Scores + mask + ONE pair-wide exp per t2 unit. Returns pts."""
            pts = []
            for t2 in t2cs:
                k0 = 128 * t2 - QT * t1
                c0 = max(0, k0)
                diag = k0 >= 0
                sp2 = ps_s.tile([128, 2 * QT], F32, name="sp2", tag="sp")
                for hi in range(HP):
                    po = hi * Dh
                    nc.tensor.matmul(
                        sp2[:, QT * hi + c0:QT * (hi + 1)],
                        lhsT=kT[p][po:po + Dh, 128 * t2:128 * (t2 + 1)],
                        rhs=qT[p][po:po + Dh, QT * t1 + c0:QT * (t1 + 1)],
                        start=True,
                        stop=not diag,
                    )
                if diag:
                    for hi in range(HP):
                        nc.tensor.matmul(
                            sp2[:, QT * hi + c0:QT * hi + c0 + 128],
                            lhsT=tri_negT,
                            rhs=ident,
                            start=False,
                            stop=True,
                        )
                pt = pp.tile([128, 2 * QT], BF16)
                src = sp2.rearrange("p (h f) -> p h f", h=HP)[:, :, c0:QT]
                dst = pt.rearrange("p (h f) -> p h f", h=HP)[:, :, c0:QT]
                nc.scalar.activation(out=dst, in_=src, func=Exp, scale=scale)
                pts.append((t2, c0, pt))
            return pts

        def emit_attnv(p, t1, pts):
            opss = [ps_o.tile([128, QT], F32, name="ops", tag="ops")
                    for _ in range(HP)]
            n = len(pts)
            for j, (t2, c0, pt) in enumerate(pts):
                for hi in range(HP):
                    h = HP * p + hi
                    nc.tensor.matmul(
                        opss[hi][0:VW, c0:QT],
                        lhsT=vaug[t2][:, VW * h:VW * (h + 1)],
                        rhs=pt[:, QT * hi + c0:QT * (hi + 1)],
                        start=(j == 0),
                        stop=(j == n - 1),
                    )
            # normalization: recip of PSUM row 0, gpsimd partition
            # broadcast (no DMA), fused evict-multiply to bf16 SBUF.
            rtss = [rtp.tile([1, QT], F32, name="rts", tag="rts")
                    for _ in range(HP)]
            for hi in range(HP):
                nc.vector.reciprocal_approx_fast(
                    out=rtss[hi], in_=opss[hi][0:1, :],
                )
            for hi in range(HP):
                po = hi * Dh
                # NOTE: partition_broadcast ignores the out partition
                # offset on hardware — each head gets its own tile so the
                # write lands at partition 0.
                rb_h = rbp.tile([Dh, QT], F32, name="rb")
                nc.gpsimd.partition_broadcast(rb_h, rtss[hi])
                nc.vector.tensor_mul(
                    out=aoutT[p][po:po + Dh, QT * t1:QT * (t1 + 1)],
                    in0=opss[hi][VZ:VW, :],
                    in1=rb_h,
                )

        def emit_outproj(eo, t1, sink=None, pool=None, tag="ps_acc"):
            box = {}

            def mm(ei):
                if ei == 0:
                    box["ps"] = (pool or ps_acc).tile(
                        [128, QT], F32, name="ps_acc", tag=tag
                    )
                nc.tensor.matmul(
                    box["ps"],
                    lhsT=wo[ei][:, 128 * eo:128 * (eo + 1)],
                    rhs=aoutT[ei][:, QT * t1:QT * (t1 + 1)],
                    start=(ei == 0),
                    stop=(ei == EC - 1),
                )

            def ev():
                nc.vector.tensor_scalar_add(
                    out=projT[eo][:, QT * t1:QT * (t1 + 1)],
                    in0=box["ps"],
                    scalar1=bo_sb[:, eo:eo + 1],
                )

            thunks = [lambda ei=ei: mm(ei) for ei in range(EC)] + [ev]
            if sink is None:
                for t in thunks:
                    t()
            else:
                sink.extend(thunks)

        store_rr = [0]
        store_engs = [nc.sync, nc.scalar, nc.gpsimd]

        def emit_ffn(ti, sink=None, pool=None, tag="ps_acc"):
            for eoq in range(NE):
                box = {}

                def mm(ei, eoq=eoq, box=box):
                    if ei == 0:
                        box["ps"] = (pool or ps_acc).tile(
                            [128, QE], F32, name="ps_acc", tag=tag
                        )
                    nc.tensor.matmul(
                        box["ps"],
                        lhsT=projT[ei][:, 128 * ti:128 * (ti + 1)],
                        rhs=w1[ei][:, QE * eoq:QE * (eoq + 1)],
                        start=(ei == 0),
                        stop=False,
                    )

                def bias(eoq=eoq, box=box):
                    nc.tensor.matmul(
                        box["ps"],
                        lhsT=ones_t[:, 0:128],
                        rhs=b1_sb[:, QE * eoq:QE * (eoq + 1)],
                        start=False,
                        stop=True,
                    )

                def ev(eoq=eoq, box=box):
                    fo = ffoutp.tile([128, QE], BF16)
                    nc.scalar.activation(out=fo, in_=box["ps"], func=Relu)
                    h0 = QE // 2
                    for s in range(2):
                        eng = store_engs[store_rr[0] % 3]
                        store_rr[0] += 1
                        eng.dma_start(
                            out=out[128 * ti:128 * (ti + 1),
                                    QE * eoq + s * h0:QE * eoq + (s + 1) * h0],
                            in_=fo[:, s * h0:(s + 1) * h0],
                        )

                thunks = [lambda ei=ei, mm=mm: mm(ei) for ei in range(EC)] + [bias, ev]
                if sink is None:
                    for t in thunks:
                        t()
                else:
                    sink.extend(thunks)

        # ================= schedule =================
        emit_xpose(range(TC // 2))
        emit_proj_half(
            wq, qT, 0, lambda out, in_: nc.vector.tensor_copy(out=out, in_=in_)
        )
        emit_proj_half(
            wk, kT, 0, lambda out, in_: nc.scalar.copy(out=out, in_=in_)
        )
        # x chunks 4-7 transposed before v (v chunk ti only needs token
        # chunk ti, but q/k half-1 fillers below need all of xT).
        emit_xpose(range(TC // 2, TC))
        # v chunks 0-3 emitted inline BEFORE the attention loop: the
        # lagged attnv(0) below must come after v3's eviction in program
        # order. The PE reaches here at ~50us, right as Wv's DMA lands.
        for ti in range(QT // 128):
            emit_vchunk(ti)

        # ---- attention t1=0 with lagged attnv and PE fillers ----
        # Fillers: q half-1 and k half-1 (no consumer inside t1=0).
        fillers = []
        emit_proj_half(
            wq, qT, 1,
            lambda out, in_: nc.vector.tensor_copy(out=out, in_=in_),
            sink=fillers,
        )
        emit_proj_half(
            wk, kT, 1,
            lambda out, in_: nc.vector.tensor_copy(out=out, in_=in_),
            sink=fillers,
        )

        t2cs0 = [t2 for t2 in range(TC) if 128 * t2 < QT]
        pending = {}
        fidx = [0]

        def drain_fillers(n):
            k = fidx[0]
            stop = min(len(fillers), k + n)
            while k < stop:
                fillers[k]()
                k += 1
            fidx[0] = k

        for p in range(NP):
            pending[p] = emit_scores(p, 0, t2cs0)
            # keep the PE fed while ACT chews on this pair's exps
            drain_fillers(18)
            if p >= LAG:
                emit_attnv(p - LAG, 0, pending.pop(p - LAG))
        drain_fillers(len(fillers))
        for p in range(NP - LAG, NP):
            emit_attnv(p, 0, pending.pop(p))

        # v chunks 4-7 (needed by attention t1=1)
        for ti in range(QT // 128, TC):
            emit_vchunk(ti)

        # ---- attention t1=1 with out-proj t1=0 interleaved ----
        t2cs1 = list(range(TC))
        fillers = []
        for eo in range(EC):
            emit_outproj(eo, 0, sink=fillers)
        fidx = [0]
        pending = {}
        for p in range(NP):
            pending[p] = emit_scores(p, 1, t2cs1)
            drain_fillers(10)
            if p >= 1:
                emit_attnv(p - 1, 1, pending.pop(p - 1))
        drain_fillers(len(fillers))
        emit_attnv(NP - 1, 1, pending.pop(NP - 1))

        # ---- FFN t1=0 chunks interleaved with out-proj t1=1 ----
        pools = [(ps_acc, "ps_acc"), (ps_s, "sp"), (ps_o, "ops")]

        def pl(i):
            return pools[i % 3]

        t1l = NT - 1
        for i, ti in enumerate(range(QT // 128)):
            po, tg = pl(i)
            emit_ffn(ti, pool=po, tag=tg)
            po, tg = pl(i + 1)
            emit_outproj(2 * i, t1l, pool=po, tag=tg)
            po, tg = pl(i + 2)
            emit_outproj(2 * i + 1, t1l, pool=po, tag=tg)
        for i, ti in enumerate(range(QT // 128, TC)):
            po, tg = pl(i)
            emit_ffn(ti, pool=po, tag=tg)

    nc.finalize()
    return nc


_NC_CACHE = {}


def _get_nc(shape_key):
    if shape_key not in _NC_CACHE:
        _NC_CACHE[shape_key] = build_nc(*shape_key)
    return _NC_CACHE[shape_key]


def kernel(x, Wq, Wk, Wv, Wo, bo, W1, b1):
    x = np.ascontiguousarray(np.asarray(x, dtype=np.float32))
    ws = {
        "Wq": np.ascontiguousarray(np.asarray(Wq, dtype=np.float32)),
        "Wk": np.ascontiguousarray(np.asarray(Wk, dtype=np.float32)),
        "Wv": np.ascontiguousarray(np.asarray(Wv, dtype=np.float32)),
        "Wo": np.ascontiguousarray(np.asarray(Wo, dtype=np.float32)),
        "bo": np.ascontiguousarray(np.asarray(bo, dtype=np.float32)),
        "W1": np.ascontiguousarray(np.asarray(W1, dtype=np.float32)),
        "b1": np.ascontiguousarray(np.asarray(b1, dtype=np.float32)),
    }
    B, TT, EE = x.shape
    assert B == N_CORES
    nc = _get_nc((TT, EE, H, DH))
    in_maps = [dict(ws, x=x[b]) for b in range(B)]
    res = run_bass_kernel_spmd(nc, in_maps, core_ids=list(range(N_CORES)))
    return np.stack(
        [np.asarray(res.results[b]["out"]) for b in range(B)], axis=0
    ).astype(np.float32)
